# revision 12
# baseline (speedup 1.0000x reference)
"""Bidirectional masked-GRU layer (PackedSequence semantics) on 8 Trainium2 cores.

Sharding: direction-split x batch-split. Cores 0-3 run the forward direction,
cores 4-7 the backward direction; within each group of 4 the batch (32) is
split into slices of 8. The backward cores run the *same* SPMD program as the
forward cores: the host pre-reverses x in time and sends per-batch active
windows [lo, hi) as data, so all per-core variation lives in the inputs.

On-device layout keeps H on the partition axis (gh^T = Whh @ h^T) so the gate
math runs on [128, 48]-shaped tiles. The recurrent matmul streams the 108
Whh^T 128x128 bf16 tiles through LDWEIGHTS each step (batch=8 moving columns),
which is the weight-ingest-bandwidth floor of a systolic array RNN.
"""

import sys

sys.path.insert(0, "/opt/trn_rl_repo")

import ml_dtypes
import numpy as np

import concourse.bacc as bacc
import concourse.bass as bass
import concourse.mybir as mybir
import concourse.tile as tile

T, B, D, H = 256, 32, 768, 768
H3 = 3 * H
KT = H // 128            # 6 contraction tiles
MT = H3 // 128           # 18 output tiles
BL = B // 4              # 8 sequences per core
GC = KT * BL             # 48 = gate tile free size (col = k*8 + b)
F32 = mybir.dt.float32
BF16 = mybir.dt.bfloat16
AFT = mybir.ActivationFunctionType
ALU = mybir.AluOpType
BF16NP = ml_dtypes.bfloat16


def build_program(t_steps: int = T, with_bhhn: bool = False):
    """Build the SPMD Bass program. Returns the compiled Bacc object."""
    nc = bacc.Bacc(None, target_bir_lowering=False, debug=False)

    ncols = t_steps * BL
    xT_d = nc.dram_tensor("xT", [D, ncols], BF16, kind="ExternalInput")
    wihT_d = nc.dram_tensor("wihT", [D, H3], BF16, kind="ExternalInput")
    whhT_d = nc.dram_tensor("whhT", [H, H3], BF16, kind="ExternalInput")
    gxb_d = nc.dram_tensor("gxbias", [128, MT], F32, kind="ExternalInput")
    h0_d = nc.dram_tensor("h0T", [128, GC], F32, kind="ExternalInput")
    lo_d = nc.dram_tensor("lo", [128, GC], F32, kind="ExternalInput")
    hi_d = nc.dram_tensor("hi", [128, GC], F32, kind="ExternalInput")
    if with_bhhn:
        bhhn_d = nc.dram_tensor("bhhn", [128, GC], F32, kind="ExternalInput")
    ost_d = nc.dram_tensor("ostage", [128, t_steps * GC], F32, kind="ExternalOutput")
    hl_d = nc.dram_tensor("hlast", [128, GC], F32, kind="ExternalOutput")

    # output staging: DMA out every SBLK steps
    SBLK = 32 if t_steps % 32 == 0 else t_steps

    with tile.TileContext(nc) as tc:
        with (
            tc.tile_pool(name="weights", bufs=1) as wpool,
            tc.tile_pool(name="bigbuf", bufs=1) as bigpool,
            tc.tile_pool(name="ps1", bufs=4, space="PSUM") as ps1,
            tc.tile_pool(name="ps2", bufs=2, space="PSUM") as ps2,
            tc.tile_pool(name="hpool", bufs=2) as hpool,
            tc.tile_pool(name="hbpool", bufs=2) as hbpool,
            tc.tile_pool(name="gates", bufs=3) as gpool,
            tc.tile_pool(name="stage", bufs=2) as stpool,
        ):
            # ---- resident loads ----
            x_sb = bigpool.tile([128, KT, ncols], BF16)
            nc.sync.dma_start(
                out=x_sb[:], in_=xT_d[:].rearrange("(k p) c -> p k c", p=128)
            )
            wih_sb = wpool.tile([128, KT, H3], BF16)
            nc.sync.dma_start(
                out=wih_sb[:], in_=wihT_d[:].rearrange("(k p) c -> p k c", p=128)
            )
            whh_sb = wpool.tile([128, KT, H3], BF16)
            nc.sync.dma_start(
                out=whh_sb[:], in_=whhT_d[:].rearrange("(k p) c -> p k c", p=128)
            )
            gxb_sb = wpool.tile([128, MT], F32)
            nc.sync.dma_start(out=gxb_sb[:], in_=gxb_d[:])
            lo_sb = wpool.tile([128, GC], F32)
            nc.sync.dma_start(out=lo_sb[:], in_=lo_d[:])
            hi_sb = wpool.tile([128, GC], F32)
            nc.sync.dma_start(out=hi_sb[:], in_=hi_d[:])
            if with_bhhn:
                bhhn_sb = wpool.tile([128, GC], F32)
                nc.sync.dma_start(out=bhhn_sb[:], in_=bhhn_d[:])

            # ---- phase 1: gx^T = Wih @ x^T  (+bias), stored bf16 ----
            # gx_sb[p, t, m*8+b] = gx[t, b, m*128+p]
            gx_sb = bigpool.tile([128, t_steps, MT * BL], BF16)
            NCH = 512
            nch = (ncols + NCH - 1) // NCH
            for m in range(MT):
                psums = []
                for c in range(nch):
                    cs = min(NCH, ncols - c * NCH)
                    psums.append(ps1.tile([128, NCH], F32, tag="ps1", name=f"ps1_{m}_{c}"))
                for k in range(KT):
                    lhsT = wih_sb[:, k, m * 128 : (m + 1) * 128]
                    for c in range(nch):
                        cs = min(NCH, ncols - c * NCH)
                        nc.tensor.matmul(
                            psums[c][:, :cs],
                            lhsT,
                            x_sb[:, k, c * NCH : c * NCH + cs],
                            start=(k == 0),
                            stop=(k == KT - 1),
                        )
                for c in range(nch):
                    cs = min(NCH, ncols - c * NCH)
                    tloc = cs // BL
                    dst = gx_sb[
                        :,
                        c * (NCH // BL) : c * (NCH // BL) + tloc,
                        m * BL : (m + 1) * BL,
                    ]
                    nc.vector.tensor_scalar(
                        dst,
                        psums[c][:, :cs].rearrange("p (t b) -> p t b", b=BL),
                        gxb_sb[:, m : m + 1],
                        None,
                        ALU.add,
                    )

            # ---- phase 2: recurrence ----
            h_prev = hpool.tile([128, GC], F32, tag="h")
            nc.sync.dma_start(out=h_prev[:], in_=h0_d[:])
            hb_prev = hbpool.tile([128, GC], BF16, tag="hb")
            nc.vector.tensor_copy(hb_prev[:], h_prev[:])

            st = None
            for i in range(t_steps):
                if i % SBLK == 0:
                    st = stpool.tile([128, SBLK, GC], F32, tag="st")

                # mask for this step: active iff lo <= i < hi  (per column)
                m1 = gpool.tile([128, GC], F32, tag="m1")
                nc.vector.tensor_scalar(m1[:], hi_sb[:], float(i), None, ALU.is_gt)
                msk = gpool.tile([128, GC], mybir.dt.uint8, tag="msk")
                nc.vector.scalar_tensor_tensor(
                    msk[:], lo_sb[:], float(i), m1[:], ALU.is_le, ALU.mult
                )
                # base copies for the masked update; emitted early so they run
                # on the DVE while the PE streams weights
                h_new = hpool.tile([128, GC], F32, tag="h")
                nc.vector.tensor_copy(h_new[:], h_prev[:])
                hb_new = hbpool.tile([128, GC], BF16, tag="hb")
                nc.vector.tensor_copy(hb_new[:], hb_prev[:])

                # r/z gates and n gate in separate PSUM tiles (= separate
                # banks): Tile tracks PSUM deps per bank, so the sigmoid can
                # start as soon as the r/z matmuls finish, overlapping the
                # n-tile matmuls on the PE.
                ghrz = ps2.tile([128, 2 * GC], F32, tag="ghrz")
                ghn = ps2.tile([128, GC], F32, tag="ghn")
                for m in range(2 * KT):
                    for k in range(KT):
                        nc.tensor.matmul(
                            ghrz[:, m * BL : (m + 1) * BL],
                            whh_sb[:, k, m * 128 : (m + 1) * 128],
                            hb_prev[:, k * BL : (k + 1) * BL],
                            start=(k == 0),
                            stop=(k == KT - 1),
                        )

                gxf = gpool.tile([128, MT * BL], F32, tag="gxf")
                nc.vector.tensor_copy(gxf[:], gx_sb[:, i, :])

                trz = gpool.tile([128, 2 * GC], F32, tag="trz")
                nc.vector.tensor_add(trz[:], ghrz[:], gxf[:, 0 : 2 * GC])
                rz = gpool.tile([128, 2 * GC], F32, tag="rz")
                nc.scalar.activation(rz[:], trz[:], AFT.Sigmoid)

                for m in range(2 * KT, MT):
                    for k in range(KT):
                        nc.tensor.matmul(
                            ghn[:, (m - 2 * KT) * BL : (m - 2 * KT + 1) * BL],
                            whh_sb[:, k, m * 128 : (m + 1) * 128],
                            hb_prev[:, k * BL : (k + 1) * BL],
                            start=(k == 0),
                            stop=(k == KT - 1),
                        )

                # z*h and (1-z) only need the sigmoid
                zh = gpool.tile([128, GC], F32, tag="zh")
                nc.vector.tensor_mul(zh[:], rz[:, GC : 2 * GC], h_prev[:])
                omz = gpool.tile([128, GC], F32, tag="omz")
                nc.vector.tensor_scalar(
                    omz[:], rz[:, GC : 2 * GC], -1.0, 1.0, ALU.mult, ALU.add
                )

                # n = tanh(gx_n + r * gh_n)
                nh = gpool.tile([128, GC], F32, tag="nh")
                if with_bhhn:
                    ghnb = gpool.tile([128, GC], F32, tag="ghnb")
                    nc.vector.tensor_add(ghnb[:], ghn[:], bhhn_sb[:])
                    nc.vector.tensor_mul(nh[:], rz[:, 0:GC], ghnb[:])
                else:
                    nc.vector.tensor_mul(nh[:], rz[:, 0:GC], ghn[:])
                nin = gpool.tile([128, GC], F32, tag="nin")
                nc.vector.tensor_add(nin[:], nh[:], gxf[:, 2 * GC :])
                nt = gpool.tile([128, GC], F32, tag="nt")
                nc.scalar.activation(nt[:], nin[:], AFT.Tanh)

                # h_cand = (1-z)*n + z*h
                p1 = gpool.tile([128, GC], F32, tag="p1")
                nc.vector.tensor_mul(p1[:], omz[:], nt[:])
                hc = gpool.tile([128, GC], F32, tag="hc")
                nc.vector.tensor_add(hc[:], p1[:], zh[:])

                # masked update (h f32 master + bf16 copy for next matmul)
                nc.vector.copy_predicated(h_new[:], msk[:], hc[:])
                nc.vector.copy_predicated(hb_new[:], msk[:], hc[:])
                nc.vector.tensor_copy(st[:, i % SBLK, :], h_new[:])
                if i % SBLK == SBLK - 1:
                    blk = i // SBLK
                    nc.sync.dma_start(
                        out=ost_d[:, blk * SBLK * GC : (blk + 1) * SBLK * GC],
                        in_=st[:].rearrange("p t c -> p (t c)"),
                    )
                h_prev, hb_prev = h_new, hb_new

            nc.sync.dma_start(out=hl_d[:], in_=h_prev[:])

    nc.compile()
    return nc


def _prep_core_inputs(x, h0, wih, whh, bih, bhh, lengths, direction, sl, t_steps):
    """Host-side input prep for one core. direction 0=fwd, 1=bwd."""
    bsl = slice(sl * BL, (sl + 1) * BL)
    xs = x[:t_steps, bsl, :]
    if direction == 1:
        xs = xs[::-1]
    xT = np.ascontiguousarray(xs.transpose(2, 0, 1).reshape(D, t_steps * BL))

    gb = bih.astype(np.float64) + bhh.astype(np.float64)
    gb[2 * H :] = bih[2 * H :]  # bhh_n stays inside the r*(.) term
    gxbias = gb.astype(np.float32).reshape(MT, 128).T

    h0s = h0[direction, bsl, :]  # [BL, H]
    h0T = np.ascontiguousarray(
        h0s.T.reshape(KT, 128, BL).transpose(1, 0, 2).reshape(128, GC)
    ).astype(np.float32)

    lens = lengths[bsl].astype(np.float32)
    if direction == 0:
        lo_v = np.zeros(BL, np.float32)
        hi_v = lens
    else:
        lo_v = t_steps - lens
        hi_v = np.full(BL, t_steps, np.float32)
    lo = np.ascontiguousarray(
        np.broadcast_to(lo_v[None, None, :], (128, KT, BL)).reshape(128, GC)
    )
    hi = np.ascontiguousarray(
        np.broadcast_to(hi_v[None, None, :], (128, KT, BL)).reshape(128, GC)
    )

    m = {
        "xT": xT.astype(BF16NP),
        "wihT": np.ascontiguousarray(wih.T).astype(BF16NP),
        "whhT": np.ascontiguousarray(whh.T).astype(BF16NP),
        "gxbias": np.ascontiguousarray(gxbias),
        "h0T": h0T,
        "lo": lo,
        "hi": hi,
    }
    bhhn = bhh[2 * H :]
    if np.any(bhhn != 0):
        m["bhhn"] = np.ascontiguousarray(
            np.broadcast_to(
                bhhn.reshape(KT, 128).T[:, :, None], (128, KT, BL)
            ).reshape(128, GC)
        ).astype(np.float32)
    return m


def make_in_maps(inputs, t_steps=T):
    x = np.asarray(inputs["x"], np.float32)
    h0 = np.asarray(inputs["h0"], np.float32)
    lengths = np.asarray(inputs["lengths"])
    maps = []
    any_bhhn = False
    for core in range(8):
        direction = core // 4
        sl = core % 4
        wih = inputs["wih_f"] if direction == 0 else inputs["wih_b"]
        whh = inputs["whh_f"] if direction == 0 else inputs["whh_b"]
        bih = inputs["bih_f"] if direction == 0 else inputs["bih_b"]
        bhh = inputs["bhh_f"] if direction == 0 else inputs["bhh_b"]
        m = _prep_core_inputs(
            x, h0,
            np.asarray(wih, np.float32), np.asarray(whh, np.float32),
            np.asarray(bih, np.float32), np.asarray(bhh, np.float32),
            lengths, direction, sl, t_steps,
        )
        any_bhhn = any_bhhn or ("bhhn" in m)
        maps.append(m)
    if any_bhhn:
        for m in maps:
            if "bhhn" not in m:
                m["bhhn"] = np.zeros((128, GC), np.float32)
    return maps, any_bhhn


def assemble_outputs(results, lengths, t_steps=T):
    out = np.zeros((t_steps, B, 2 * H), np.float32)
    h_out = np.zeros((2, B, H), np.float32)
    for core in range(8):
        direction = core // 4
        sl = core % 4
        bsl = slice(sl * BL, (sl + 1) * BL)
        stage = results[core]["ostage"].reshape(128, t_steps, KT, BL)
        if direction == 1:
            stage = stage[:, ::-1]
        # out[t, b, k*128+p] = stage[p, t, k, b]
        blockv = stage.transpose(1, 3, 2, 0).reshape(t_steps, BL, H)
        out[:, bsl, direction * H : (direction + 1) * H] = blockv
        hl = results[core]["hlast"].reshape(128, KT, BL)
        h_out[direction, bsl, :] = hl.transpose(2, 1, 0).reshape(BL, H)
    tmask = np.arange(t_steps)[:, None] >= np.asarray(lengths)[None, :]
    out[tmask] = 0.0
    return out, h_out


_prog_cache = {}


def kernel(**inputs):
    from concourse.bass_utils import run_bass_kernel_spmd

    maps, any_bhhn = make_in_maps(inputs)
    key = (T, any_bhhn)
    if key not in _prog_cache:
        _prog_cache[key] = build_program(T, any_bhhn)
    nc = _prog_cache[key]
    res = run_bass_kernel_spmd(nc, maps, list(range(8)))
    out, h_out = assemble_outputs(res.results, inputs["lengths"])
    return out, h_out


# revision 22
# speedup vs baseline: 11.7673x; 11.7673x over previous
"""Bidirectional masked-GRU layer (PackedSequence semantics) on 8 Trainium2 cores.

Sharding: direction-split x batch-split. Cores 0-3 run the forward direction,
cores 4-7 the backward direction; within each group of 4 the batch (32) is
split into slices of 8. The backward cores run the *same* SPMD program as the
forward cores: the host pre-reverses x in time and sends per-batch active
windows [lo, hi) as data, so all per-core variation lives in the inputs.

On-device layout keeps H on the partition axis (gh^T = Whh @ h^T) so the gate
math runs on [128, 48]-shaped tiles. The recurrent matmul streams the 108
Whh^T 128x128 bf16 tiles through LDWEIGHTS each step (batch=8 moving columns),
which is the weight-ingest-bandwidth floor of a systolic array RNN.
"""

import sys

sys.path.insert(0, "/opt/trn_rl_repo")

import ml_dtypes
import numpy as np

import concourse.bacc as bacc
import concourse.bass as bass
import concourse.mybir as mybir
import concourse.tile as tile

T, B, D, H = 256, 32, 768, 768
H3 = 3 * H
KT = H // 128            # 6 contraction tiles
MT = H3 // 128           # 18 output tiles
BL = B // 4              # 8 sequences per core
GC = KT * BL             # 48 = gate tile free size (col = k*8 + b)
F32 = mybir.dt.float32
BF16 = mybir.dt.bfloat16
AFT = mybir.ActivationFunctionType
ALU = mybir.AluOpType
BF16NP = ml_dtypes.bfloat16


def build_program(t_steps: int = T, with_bhhn: bool = False, reps: int = 1):
    """Build the SPMD Bass program. Returns the compiled Bacc object.

    reps > 1 wraps the whole kernel body in a hardware loop that repeats it
    (identical work each iteration) — used only for timing amplification.
    """
    nc = bacc.Bacc(None, target_bir_lowering=False, debug=False)

    ncols = t_steps * BL
    xT_d = nc.dram_tensor("xT", [D, ncols], BF16, kind="ExternalInput")
    wihT_d = nc.dram_tensor("wihT", [D, H3], BF16, kind="ExternalInput")
    whhT_d = nc.dram_tensor("whhT", [H, H3], BF16, kind="ExternalInput")
    gxb_d = nc.dram_tensor("gxbias", [128, MT], F32, kind="ExternalInput")
    h0_d = nc.dram_tensor("h0T", [128, GC], F32, kind="ExternalInput")
    lo_d = nc.dram_tensor("lo", [128, GC], F32, kind="ExternalInput")
    hi_d = nc.dram_tensor("hi", [128, GC], F32, kind="ExternalInput")
    if with_bhhn:
        bhhn_d = nc.dram_tensor("bhhn", [128, GC], F32, kind="ExternalInput")
    ost_d = nc.dram_tensor("ostage", [128, t_steps * GC], F32, kind="ExternalOutput")
    hl_d = nc.dram_tensor("hlast", [128, GC], F32, kind="ExternalOutput")

    # output staging: DMA out every SBLK steps
    SBLK = 32 if t_steps % 32 == 0 else t_steps

    with tile.TileContext(nc) as tc:
        with (
            tc.tile_pool(name="weights", bufs=1) as wpool,
            tc.tile_pool(name="bigbuf", bufs=1) as bigpool,
            tc.tile_pool(name="ps1", bufs=2, space="PSUM") as ps1,
            tc.tile_pool(name="ps2", bufs=2, space="PSUM") as ps2,
            tc.tile_pool(name="hpool", bufs=2) as hpool,
            tc.tile_pool(name="hbpool", bufs=2) as hbpool,
            tc.tile_pool(name="gates", bufs=3) as gpool,
            tc.tile_pool(name="stage", bufs=2) as stpool,
        ):
            import contextlib

            rep_ctx = (
                tc.For_i(0, reps, 1) if reps > 1 else contextlib.nullcontext()
            )
            with rep_ctx:
                _kernel_body(
                    nc, tc, t_steps, with_bhhn, locals_dict=dict(
                        xT_d=xT_d, wihT_d=wihT_d, whhT_d=whhT_d, gxb_d=gxb_d,
                        h0_d=h0_d, lo_d=lo_d, hi_d=hi_d,
                        bhhn_d=bhhn_d if with_bhhn else None,
                        ost_d=ost_d, hl_d=hl_d, SBLK=SBLK,
                        wpool=wpool, bigpool=bigpool, ps1=ps1, ps2=ps2,
                        hpool=hpool, hbpool=hbpool, gpool=gpool, stpool=stpool,
                    ),
                )

    nc.compile()
    return nc


def _kernel_body(nc, tc, t_steps, with_bhhn, locals_dict):
    ld = locals_dict
    xT_d, wihT_d, whhT_d = ld["xT_d"], ld["wihT_d"], ld["whhT_d"]
    gxb_d, h0_d, lo_d, hi_d = ld["gxb_d"], ld["h0_d"], ld["lo_d"], ld["hi_d"]
    bhhn_d, ost_d, hl_d, SBLK = ld["bhhn_d"], ld["ost_d"], ld["hl_d"], ld["SBLK"]
    wpool, bigpool, ps1, ps2 = ld["wpool"], ld["bigpool"], ld["ps1"], ld["ps2"]
    hpool, hbpool, gpool, stpool = (
        ld["hpool"], ld["hbpool"], ld["gpool"], ld["stpool"],
    )
    ncols = t_steps * BL
    if True:
            # ---- resident loads ----
            x_sb = bigpool.tile([128, KT, ncols], BF16)
            nc.sync.dma_start(
                out=x_sb[:], in_=xT_d[:].rearrange("(k p) c -> p k c", p=128)
            )
            wih_sb = wpool.tile([128, KT, H3], BF16)
            nc.sync.dma_start(
                out=wih_sb[:], in_=wihT_d[:].rearrange("(k p) c -> p k c", p=128)
            )
            whh_sb = wpool.tile([128, KT, H3], BF16)
            nc.sync.dma_start(
                out=whh_sb[:], in_=whhT_d[:].rearrange("(k p) c -> p k c", p=128)
            )
            gxb_sb = wpool.tile([128, MT], F32)
            nc.sync.dma_start(out=gxb_sb[:], in_=gxb_d[:])
            lo_sb = wpool.tile([128, GC], F32)
            nc.sync.dma_start(out=lo_sb[:], in_=lo_d[:])
            hi_sb = wpool.tile([128, GC], F32)
            nc.sync.dma_start(out=hi_sb[:], in_=hi_d[:])
            if with_bhhn:
                bhhn_sb = wpool.tile([128, GC], F32)
                nc.sync.dma_start(out=bhhn_sb[:], in_=bhhn_d[:])

            # ---- phase 1: gx^T = Wih @ x^T  (+bias), stored bf16 ----
            # gx_sb[p, t, m*8+b] = gx[t, b, m*128+p]
            gx_sb = bigpool.tile([128, t_steps, MT * BL], BF16)
            NCH = 512
            nch = (ncols + NCH - 1) // NCH
            # chunk-outer ordering: one live PSUM chunk (+1 for overlap);
            # per-chunk LDWEIGHTS reloads hide under the N=512 streaming.
            for m in range(MT):
                for c in range(nch):
                    cs = min(NCH, ncols - c * NCH)
                    ps = ps1.tile([128, NCH], F32, tag="ps1", name=f"ps1_{m}_{c}")
                    for k in range(KT):
                        nc.tensor.matmul(
                            ps[:, :cs],
                            wih_sb[:, k, m * 128 : (m + 1) * 128],
                            x_sb[:, k, c * NCH : c * NCH + cs],
                            start=(k == 0),
                            stop=(k == KT - 1),
                        )
                    tloc = cs // BL
                    dst = gx_sb[
                        :,
                        c * (NCH // BL) : c * (NCH // BL) + tloc,
                        m * BL : (m + 1) * BL,
                    ]
                    nc.vector.tensor_scalar(
                        dst,
                        ps[:, :cs].rearrange("p (t b) -> p t b", b=BL),
                        gxb_sb[:, m : m + 1],
                        None,
                        ALU.add,
                    )

            # ---- phase 2: recurrence ----
            h_prev = hpool.tile([128, GC], F32, tag="h")
            nc.sync.dma_start(out=h_prev[:], in_=h0_d[:])
            hb_prev = hbpool.tile([128, GC], BF16, tag="hb")
            nc.vector.tensor_copy(hb_prev[:], h_prev[:])

            st = None
            for i in range(t_steps):
                if i % SBLK == 0:
                    st = stpool.tile([128, SBLK, GC], F32, tag="st")

                # mask for this step: active iff lo <= i < hi  (per column)
                m1 = gpool.tile([128, GC], F32, tag="m1")
                nc.vector.tensor_scalar(m1[:], hi_sb[:], float(i), None, ALU.is_gt)
                msk = gpool.tile([128, GC], mybir.dt.uint8, tag="msk")
                nc.vector.scalar_tensor_tensor(
                    msk[:], lo_sb[:], float(i), m1[:], ALU.is_le, ALU.mult
                )
                # base copies for the masked update; emitted early so they run
                # on the DVE while the PE streams weights
                h_new = hpool.tile([128, GC], F32, tag="h")
                nc.vector.tensor_copy(h_new[:], h_prev[:])
                hb_new = hbpool.tile([128, GC], BF16, tag="hb")
                nc.vector.tensor_copy(hb_new[:], hb_prev[:])

                # r/z gates in one PSUM bank, the n gate split across two more
                # banks: Tile tracks PSUM deps per bank, so the sigmoid starts
                # as soon as the r/z matmuls finish (overlapping the n-tile
                # matmuls), and the first-half n-gate math overlaps the
                # second-half n matmuls.
                ghrz = ps2.tile([128, 2 * GC], F32, tag="ghrz")
                ghna = ps2.tile([128, GC // 2], F32, tag="ghna")
                ghnb = ps2.tile([128, GC // 2], F32, tag="ghnb")
                for m in range(2 * KT):
                    for k in range(KT):
                        nc.tensor.matmul(
                            ghrz[:, m * BL : (m + 1) * BL],
                            whh_sb[:, k, m * 128 : (m + 1) * 128],
                            hb_prev[:, k * BL : (k + 1) * BL],
                            start=(k == 0),
                            stop=(k == KT - 1),
                        )

                gxf = gpool.tile([128, MT * BL], F32, tag="gxf")
                nc.vector.tensor_copy(gxf[:], gx_sb[:, i, :])

                trz = gpool.tile([128, 2 * GC], F32, tag="trz")
                nc.vector.tensor_add(trz[:], ghrz[:], gxf[:, 0 : 2 * GC])
                rz = gpool.tile([128, 2 * GC], F32, tag="rz")
                nc.scalar.activation(rz[:], trz[:], AFT.Sigmoid)

                HGC = GC // 2
                for m in range(2 * KT, MT):
                    ghh_t = ghna if m < 2 * KT + KT // 2 else ghnb
                    mo = (m - 2 * KT) % (KT // 2)
                    for k in range(KT):
                        nc.tensor.matmul(
                            ghh_t[:, mo * BL : (mo + 1) * BL],
                            whh_sb[:, k, m * 128 : (m + 1) * 128],
                            hb_prev[:, k * BL : (k + 1) * BL],
                            start=(k == 0),
                            stop=(k == KT - 1),
                        )

                # z*h and (1-z) only need the sigmoid; they run on the DVE
                # while the PE streams the n-tile weights
                zh = gpool.tile([128, GC], F32, tag="zh")
                nc.vector.tensor_mul(zh[:], rz[:, GC : 2 * GC], h_prev[:])
                omz = gpool.tile([128, GC], F32, tag="omz")
                nc.vector.tensor_scalar(
                    omz[:], rz[:, GC : 2 * GC], -1.0, 1.0, ALU.mult, ALU.add
                )
                zho = gpool.tile([128, GC], F32, tag="zho")
                nc.vector.tensor_sub(zho[:], zh[:], omz[:])

                # n = tanh(gx_n + r*gh_n), tanh(x) = 2*sigmoid(2x) - 1 (keeps
                # the ACT engine on one function table). Done in two column
                # halves so the first half overlaps the second half's matmuls.
                nh = gpool.tile([128, GC], F32, tag="nh")
                nin = gpool.tile([128, GC], F32, tag="nin")
                nt = gpool.tile([128, GC], F32, tag="nt")
                p1 = gpool.tile([128, GC], F32, tag="p1")
                hc = gpool.tile([128, GC], F32, tag="hc")
                for half, ghh in ((0, ghna), (1, ghnb)):
                    sl = slice(half * HGC, (half + 1) * HGC)
                    if with_bhhn:
                        ghb = gpool.tile([128, HGC], F32, tag=f"ghb{half}",
                                         name=f"ghb_{half}")
                        nc.vector.tensor_add(ghb[:], ghh[:], bhhn_sb[:, sl])
                        nc.vector.tensor_mul(nh[:, sl], rz[:, sl], ghb[:])
                    else:
                        nc.vector.tensor_mul(nh[:, sl], rz[:, sl], ghh[:])
                    nc.vector.tensor_add(
                        nin[:, sl], nh[:, sl], gxf[:, 2 * GC + half * HGC :
                                                   2 * GC + (half + 1) * HGC]
                    )
                    nc.scalar.activation(
                        nt[:, sl], nin[:, sl], AFT.Sigmoid, scale=2.0
                    )
                    # h_cand = (1-z)*n + z*h = 2*(1-z)*s + (z*h - (1-z))
                    nc.vector.tensor_mul(p1[:, sl], omz[:, sl], nt[:, sl])
                    nc.vector.scalar_tensor_tensor(
                        hc[:, sl], p1[:, sl], 2.0, zho[:, sl], ALU.mult, ALU.add
                    )
                    nc.vector.copy_predicated(h_new[:, sl], msk[:, sl], hc[:, sl])
                    nc.vector.copy_predicated(hb_new[:, sl], msk[:, sl], hc[:, sl])
                nc.vector.tensor_copy(st[:, i % SBLK, :], h_new[:])
                if i % SBLK == SBLK - 1:
                    blk = i // SBLK
                    nc.sync.dma_start(
                        out=ost_d[:, blk * SBLK * GC : (blk + 1) * SBLK * GC],
                        in_=st[:].rearrange("p t c -> p (t c)"),
                    )
                h_prev, hb_prev = h_new, hb_new

            nc.sync.dma_start(out=hl_d[:], in_=h_prev[:])


def _prep_core_inputs(x, h0, wih, whh, bih, bhh, lengths, direction, sl, t_steps):
    """Host-side input prep for one core. direction 0=fwd, 1=bwd."""
    bsl = slice(sl * BL, (sl + 1) * BL)
    xs = x[:t_steps, bsl, :]
    if direction == 1:
        xs = xs[::-1]
    xT = np.ascontiguousarray(xs.transpose(2, 0, 1).reshape(D, t_steps * BL))

    gb = bih.astype(np.float64) + bhh.astype(np.float64)
    gb[2 * H :] = bih[2 * H :]  # bhh_n stays inside the r*(.) term
    gxbias = gb.astype(np.float32).reshape(MT, 128).T

    h0s = h0[direction, bsl, :]  # [BL, H]
    h0T = np.ascontiguousarray(
        h0s.T.reshape(KT, 128, BL).transpose(1, 0, 2).reshape(128, GC)
    ).astype(np.float32)

    lens = lengths[bsl].astype(np.float32)
    if direction == 0:
        lo_v = np.zeros(BL, np.float32)
        hi_v = lens
    else:
        lo_v = t_steps - lens
        hi_v = np.full(BL, t_steps, np.float32)
    lo = np.ascontiguousarray(
        np.broadcast_to(lo_v[None, None, :], (128, KT, BL)).reshape(128, GC)
    )
    hi = np.ascontiguousarray(
        np.broadcast_to(hi_v[None, None, :], (128, KT, BL)).reshape(128, GC)
    )

    m = {
        "xT": xT.astype(BF16NP),
        "wihT": np.ascontiguousarray(wih.T).astype(BF16NP),
        "whhT": np.ascontiguousarray(whh.T).astype(BF16NP),
        "gxbias": np.ascontiguousarray(gxbias),
        "h0T": h0T,
        "lo": lo,
        "hi": hi,
    }
    bhhn = bhh[2 * H :]
    if np.any(bhhn != 0):
        m["bhhn"] = np.ascontiguousarray(
            np.broadcast_to(
                bhhn.reshape(KT, 128).T[:, :, None], (128, KT, BL)
            ).reshape(128, GC)
        ).astype(np.float32)
    return m


def make_in_maps(inputs, t_steps=T):
    x = np.asarray(inputs["x"], np.float32)
    h0 = np.asarray(inputs["h0"], np.float32)
    lengths = np.asarray(inputs["lengths"])
    maps = []
    any_bhhn = False
    for core in range(8):
        direction = core // 4
        sl = core % 4
        wih = inputs["wih_f"] if direction == 0 else inputs["wih_b"]
        whh = inputs["whh_f"] if direction == 0 else inputs["whh_b"]
        bih = inputs["bih_f"] if direction == 0 else inputs["bih_b"]
        bhh = inputs["bhh_f"] if direction == 0 else inputs["bhh_b"]
        m = _prep_core_inputs(
            x, h0,
            np.asarray(wih, np.float32), np.asarray(whh, np.float32),
            np.asarray(bih, np.float32), np.asarray(bhh, np.float32),
            lengths, direction, sl, t_steps,
        )
        any_bhhn = any_bhhn or ("bhhn" in m)
        maps.append(m)
    if any_bhhn:
        for m in maps:
            if "bhhn" not in m:
                m["bhhn"] = np.zeros((128, GC), np.float32)
    return maps, any_bhhn


def assemble_outputs(results, lengths, t_steps=T):
    out = np.zeros((t_steps, B, 2 * H), np.float32)
    h_out = np.zeros((2, B, H), np.float32)
    for core in range(8):
        direction = core // 4
        sl = core % 4
        bsl = slice(sl * BL, (sl + 1) * BL)
        stage = results[core]["ostage"].reshape(128, t_steps, KT, BL)
        if direction == 1:
            stage = stage[:, ::-1]
        # out[t, b, k*128+p] = stage[p, t, k, b]
        blockv = stage.transpose(1, 3, 2, 0).reshape(t_steps, BL, H)
        out[:, bsl, direction * H : (direction + 1) * H] = blockv
        hl = results[core]["hlast"].reshape(128, KT, BL)
        h_out[direction, bsl, :] = hl.transpose(2, 1, 0).reshape(BL, H)
    tmask = np.arange(t_steps)[:, None] >= np.asarray(lengths)[None, :]
    out[tmask] = 0.0
    return out, h_out


_prog_cache = {}


def kernel(**inputs):
    from concourse.bass_utils import run_bass_kernel_spmd

    maps, any_bhhn = make_in_maps(inputs)
    key = (T, any_bhhn)
    if key not in _prog_cache:
        _prog_cache[key] = build_program(T, any_bhhn)
    nc = _prog_cache[key]
    res = run_bass_kernel_spmd(nc, maps, list(range(8)))
    out, h_out = assemble_outputs(res.results, inputs["lengths"])
    return out, h_out


# revision 25
# speedup vs baseline: 12.4694x; 1.0597x over previous
"""Bidirectional masked-GRU layer (PackedSequence semantics) on 8 Trainium2 cores.

Sharding: direction-split x batch-split. Cores 0-3 run the forward direction,
cores 4-7 the backward direction; within each group of 4 the batch (32) is
split into slices of 8. The backward cores run the *same* SPMD program as the
forward cores: the host pre-reverses x in time and sends per-batch active
windows [lo, hi) as data, so all per-core variation lives in the inputs.

On-device layout keeps H on the partition axis (gh^T = Whh @ h^T) so the gate
math runs on [128, 48]-shaped tiles. The recurrent matmul streams the 108
Whh^T 128x128 bf16 tiles through LDWEIGHTS each step (batch=8 moving columns),
which is the weight-ingest-bandwidth floor of a systolic array RNN.
"""

import sys

sys.path.insert(0, "/opt/trn_rl_repo")

import ml_dtypes
import numpy as np

import concourse.bacc as bacc
import concourse.bass as bass
import concourse.mybir as mybir
import concourse.tile as tile

T, B, D, H = 256, 32, 768, 768
H3 = 3 * H
KT = H // 128            # 6 contraction tiles
MT = H3 // 128           # 18 output tiles
BL = B // 4              # 8 sequences per core
GC = KT * BL             # 48 = gate tile free size (col = k*8 + b)
F32 = mybir.dt.float32
BF16 = mybir.dt.bfloat16
AFT = mybir.ActivationFunctionType
ALU = mybir.AluOpType
BF16NP = ml_dtypes.bfloat16


def build_program(t_steps: int = T, with_bhhn: bool = False, reps: int = 1):
    """Build the SPMD Bass program. Returns the compiled Bacc object.

    reps > 1 wraps the whole kernel body in a hardware loop that repeats it
    (identical work each iteration) — used only for timing amplification.
    """
    nc = bacc.Bacc(None, target_bir_lowering=False, debug=False)

    ncols = t_steps * BL
    xT_d = nc.dram_tensor("xT", [D, ncols], BF16, kind="ExternalInput")
    wihT_d = nc.dram_tensor("wihT", [D, H3], BF16, kind="ExternalInput")
    whhT_d = nc.dram_tensor("whhT", [H, H3], BF16, kind="ExternalInput")
    gxb_d = nc.dram_tensor("gxbias", [128, MT], F32, kind="ExternalInput")
    h0_d = nc.dram_tensor("h0T", [128, GC], F32, kind="ExternalInput")
    lo_d = nc.dram_tensor("lo", [128, GC], F32, kind="ExternalInput")
    hi_d = nc.dram_tensor("hi", [128, GC], F32, kind="ExternalInput")
    if with_bhhn:
        bhhn_d = nc.dram_tensor("bhhn", [128, GC], F32, kind="ExternalInput")
    ost_d = nc.dram_tensor("ostage", [128, t_steps * GC], F32, kind="ExternalOutput")
    hl_d = nc.dram_tensor("hlast", [128, GC], F32, kind="ExternalOutput")

    # output staging: DMA out every SBLK steps
    SBLK = next((s for s in (32, 16, 8) if t_steps % s == 0), t_steps)

    with tile.TileContext(nc) as tc:
        with (
            tc.tile_pool(name="weights", bufs=1) as wpool,
            tc.tile_pool(name="bigbuf", bufs=1) as bigpool,
            tc.tile_pool(name="ps1", bufs=2, space="PSUM") as ps1,
            tc.tile_pool(name="ps2", bufs=2, space="PSUM") as ps2,
            tc.tile_pool(name="hpool", bufs=2) as hpool,
            tc.tile_pool(name="hbpool", bufs=2) as hbpool,
            tc.tile_pool(name="gates", bufs=3) as gpool,
            tc.tile_pool(name="stage", bufs=2) as stpool,
        ):
            import contextlib

            rep_ctx = (
                tc.For_i(0, reps, 1) if reps > 1 else contextlib.nullcontext()
            )
            with rep_ctx:
                _kernel_body(
                    nc, tc, t_steps, with_bhhn, locals_dict=dict(
                        xT_d=xT_d, wihT_d=wihT_d, whhT_d=whhT_d, gxb_d=gxb_d,
                        h0_d=h0_d, lo_d=lo_d, hi_d=hi_d,
                        bhhn_d=bhhn_d if with_bhhn else None,
                        ost_d=ost_d, hl_d=hl_d, SBLK=SBLK,
                        wpool=wpool, bigpool=bigpool, ps1=ps1, ps2=ps2,
                        hpool=hpool, hbpool=hbpool, gpool=gpool, stpool=stpool,
                    ),
                )

    nc.compile()
    return nc


def _kernel_body(nc, tc, t_steps, with_bhhn, locals_dict):
    ld = locals_dict
    xT_d, wihT_d, whhT_d = ld["xT_d"], ld["wihT_d"], ld["whhT_d"]
    gxb_d, h0_d, lo_d, hi_d = ld["gxb_d"], ld["h0_d"], ld["lo_d"], ld["hi_d"]
    bhhn_d, ost_d, hl_d, SBLK = ld["bhhn_d"], ld["ost_d"], ld["hl_d"], ld["SBLK"]
    wpool, bigpool, ps1, ps2 = ld["wpool"], ld["bigpool"], ld["ps1"], ld["ps2"]
    hpool, hbpool, gpool, stpool = (
        ld["hpool"], ld["hbpool"], ld["gpool"], ld["stpool"],
    )
    ncols = t_steps * BL
    if True:
            # ---- resident loads ----
            x_sb = bigpool.tile([128, KT, ncols], BF16)
            nc.sync.dma_start(
                out=x_sb[:], in_=xT_d[:].rearrange("(k p) c -> p k c", p=128)
            )
            wih_sb = wpool.tile([128, KT, H3], BF16)
            nc.sync.dma_start(
                out=wih_sb[:], in_=wihT_d[:].rearrange("(k p) c -> p k c", p=128)
            )
            whh_sb = wpool.tile([128, KT, H3], BF16)
            nc.sync.dma_start(
                out=whh_sb[:], in_=whhT_d[:].rearrange("(k p) c -> p k c", p=128)
            )
            gxb_sb = wpool.tile([128, MT], F32)
            nc.sync.dma_start(out=gxb_sb[:], in_=gxb_d[:])
            lo_sb = wpool.tile([128, GC], F32)
            nc.sync.dma_start(out=lo_sb[:], in_=lo_d[:])
            hi_sb = wpool.tile([128, GC], F32)
            nc.sync.dma_start(out=hi_sb[:], in_=hi_d[:])
            if with_bhhn:
                bhhn_sb = wpool.tile([128, GC], F32)
                nc.sync.dma_start(out=bhhn_sb[:], in_=bhhn_d[:])

            # ---- phase 1: gx^T = Wih @ x^T  (+bias), stored bf16 ----
            # gx_sb[p, t, m*8+b] = gx[t, b, m*128+p]
            gx_sb = bigpool.tile([128, t_steps, MT * BL], BF16)
            NCH = 512
            nch = (ncols + NCH - 1) // NCH
            # chunk-outer ordering: one live PSUM chunk (+1 for overlap);
            # per-chunk LDWEIGHTS reloads hide under the N=512 streaming.
            for m in range(MT):
                for c in range(nch):
                    cs = min(NCH, ncols - c * NCH)
                    ps = ps1.tile([128, NCH], F32, tag="ps1", name=f"ps1_{m}_{c}")
                    for k in range(KT):
                        nc.tensor.matmul(
                            ps[:, :cs],
                            wih_sb[:, k, m * 128 : (m + 1) * 128],
                            x_sb[:, k, c * NCH : c * NCH + cs],
                            start=(k == 0),
                            stop=(k == KT - 1),
                        )
                    tloc = cs // BL
                    dst = gx_sb[
                        :,
                        c * (NCH // BL) : c * (NCH // BL) + tloc,
                        m * BL : (m + 1) * BL,
                    ]
                    nc.vector.tensor_scalar(
                        dst,
                        ps[:, :cs].rearrange("p (t b) -> p t b", b=BL),
                        gxb_sb[:, m : m + 1],
                        None,
                        ALU.add,
                    )

            # ---- phase 2: recurrence ----
            h_prev = hpool.tile([128, GC], F32, tag="h")
            nc.sync.dma_start(out=h_prev[:], in_=h0_d[:])
            hb_prev = hbpool.tile([128, GC], BF16, tag="hb")
            nc.vector.tensor_copy(hb_prev[:], h_prev[:])

            st = None
            for i in range(t_steps):
                if i % SBLK == 0:
                    st = stpool.tile([128, SBLK, GC], F32, tag="st")

                # mask for this step: active iff lo <= i < hi  (per column)
                m1 = gpool.tile([128, GC], F32, tag="m1")
                nc.vector.tensor_scalar(m1[:], hi_sb[:], float(i), None, ALU.is_gt)
                msk = gpool.tile([128, GC], mybir.dt.uint8, tag="msk")
                nc.vector.scalar_tensor_tensor(
                    msk[:], lo_sb[:], float(i), m1[:], ALU.is_le, ALU.mult
                )
                # base copies for the masked update; emitted early so they run
                # on the DVE while the PE streams weights
                h_new = hpool.tile([128, GC], F32, tag="h")
                nc.vector.tensor_copy(h_new[:], h_prev[:])
                hb_new = hbpool.tile([128, GC], BF16, tag="hb")
                nc.vector.tensor_copy(hb_new[:], hb_prev[:])

                # r/z gates in one PSUM bank, the n gate split across two more
                # banks: Tile tracks PSUM deps per bank, so the sigmoid starts
                # as soon as the r/z matmuls finish (overlapping the n-tile
                # matmuls), and the first-half n-gate math overlaps the
                # second-half n matmuls.
                ghrz = ps2.tile([128, 2 * GC], F32, tag="ghrz")
                ghna = ps2.tile([128, GC // 2], F32, tag="ghna")
                ghnb = ps2.tile([128, GC // 2], F32, tag="ghnb")
                for m in range(2 * KT):
                    for k in range(KT):
                        nc.tensor.matmul(
                            ghrz[:, m * BL : (m + 1) * BL],
                            whh_sb[:, k, m * 128 : (m + 1) * 128],
                            hb_prev[:, k * BL : (k + 1) * BL],
                            start=(k == 0),
                            stop=(k == KT - 1),
                        )

                gxf = gpool.tile([128, MT * BL], F32, tag="gxf")
                nc.vector.tensor_copy(gxf[:], gx_sb[:, i, :])

                trz = gpool.tile([128, 2 * GC], F32, tag="trz")
                nc.vector.tensor_add(trz[:], ghrz[:], gxf[:, 0 : 2 * GC])
                rz = gpool.tile([128, 2 * GC], F32, tag="rz")
                nc.scalar.activation(rz[:], trz[:], AFT.Sigmoid)

                HGC = GC // 2
                for m in range(2 * KT, MT):
                    ghh_t = ghna if m < 2 * KT + KT // 2 else ghnb
                    mo = (m - 2 * KT) % (KT // 2)
                    for k in range(KT):
                        nc.tensor.matmul(
                            ghh_t[:, mo * BL : (mo + 1) * BL],
                            whh_sb[:, k, m * 128 : (m + 1) * 128],
                            hb_prev[:, k * BL : (k + 1) * BL],
                            start=(k == 0),
                            stop=(k == KT - 1),
                        )

                # z*h and (1-z) only need the sigmoid; they run on the DVE
                # while the PE streams the n-tile weights
                zh = gpool.tile([128, GC], F32, tag="zh")
                nc.vector.tensor_mul(zh[:], rz[:, GC : 2 * GC], h_prev[:])
                omz = gpool.tile([128, GC], F32, tag="omz")
                nc.vector.tensor_scalar(
                    omz[:], rz[:, GC : 2 * GC], -1.0, 1.0, ALU.mult, ALU.add
                )
                zho = gpool.tile([128, GC], F32, tag="zho")
                nc.vector.tensor_sub(zho[:], zh[:], omz[:])

                # n = tanh(gx_n + r*gh_n), tanh(x) = 2*sigmoid(2x) - 1 (keeps
                # the ACT engine on one function table). Done in two column
                # halves so the first half overlaps the second half's matmuls.
                nh = gpool.tile([128, GC], F32, tag="nh")
                nin = gpool.tile([128, GC], F32, tag="nin")
                nt = gpool.tile([128, GC], F32, tag="nt")
                p1 = gpool.tile([128, GC], F32, tag="p1")
                hc = gpool.tile([128, GC], F32, tag="hc")
                for half, ghh in ((0, ghna), (1, ghnb)):
                    sl = slice(half * HGC, (half + 1) * HGC)
                    if with_bhhn:
                        ghb = gpool.tile([128, HGC], F32, tag=f"ghb{half}",
                                         name=f"ghb_{half}")
                        nc.vector.tensor_add(ghb[:], ghh[:], bhhn_sb[:, sl])
                        nc.vector.tensor_mul(nh[:, sl], rz[:, sl], ghb[:])
                    else:
                        nc.vector.tensor_mul(nh[:, sl], rz[:, sl], ghh[:])
                    nc.vector.tensor_add(
                        nin[:, sl], nh[:, sl], gxf[:, 2 * GC + half * HGC :
                                                   2 * GC + (half + 1) * HGC]
                    )
                    nc.scalar.activation(
                        nt[:, sl], nin[:, sl], AFT.Sigmoid, scale=2.0
                    )
                    # h_cand = (1-z)*n + z*h = 2*(1-z)*s + (z*h - (1-z))
                    nc.vector.tensor_mul(p1[:, sl], omz[:, sl], nt[:, sl])
                    nc.vector.scalar_tensor_tensor(
                        hc[:, sl], p1[:, sl], 2.0, zho[:, sl], ALU.mult, ALU.add
                    )
                    nc.vector.copy_predicated(h_new[:, sl], msk[:, sl], hc[:, sl])
                    nc.vector.copy_predicated(hb_new[:, sl], msk[:, sl], hc[:, sl])
                nc.vector.tensor_copy(st[:, i % SBLK, :], h_new[:])
                if i % SBLK == SBLK - 1:
                    blk = i // SBLK
                    nc.sync.dma_start(
                        out=ost_d[:, blk * SBLK * GC : (blk + 1) * SBLK * GC],
                        in_=st[:].rearrange("p t c -> p (t c)"),
                    )
                h_prev, hb_prev = h_new, hb_new

            nc.sync.dma_start(out=hl_d[:], in_=h_prev[:])


def _prep_core_inputs(x, h0, wih, whh, bih, bhh, lengths, direction, sl, t_steps):
    """Host-side input prep for one core. direction 0=fwd, 1=bwd."""
    bsl = slice(sl * BL, (sl + 1) * BL)
    xs = x[:t_steps, bsl, :]
    if direction == 1:
        xs = xs[::-1]
    xT = np.ascontiguousarray(xs.transpose(2, 0, 1).reshape(D, t_steps * BL))

    gb = bih.astype(np.float64) + bhh.astype(np.float64)
    gb[2 * H :] = bih[2 * H :]  # bhh_n stays inside the r*(.) term
    gxbias = gb.astype(np.float32).reshape(MT, 128).T

    h0s = h0[direction, bsl, :]  # [BL, H]
    h0T = np.ascontiguousarray(
        h0s.T.reshape(KT, 128, BL).transpose(1, 0, 2).reshape(128, GC)
    ).astype(np.float32)

    lens = lengths[bsl].astype(np.float32)
    if direction == 0:
        lo_v = np.zeros(BL, np.float32)
        hi_v = lens
    else:
        lo_v = t_steps - lens
        hi_v = np.full(BL, t_steps, np.float32)
    lo = np.ascontiguousarray(
        np.broadcast_to(lo_v[None, None, :], (128, KT, BL)).reshape(128, GC)
    )
    hi = np.ascontiguousarray(
        np.broadcast_to(hi_v[None, None, :], (128, KT, BL)).reshape(128, GC)
    )

    m = {
        "xT": xT.astype(BF16NP),
        "wihT": np.ascontiguousarray(wih.T).astype(BF16NP),
        "whhT": np.ascontiguousarray(whh.T).astype(BF16NP),
        "gxbias": np.ascontiguousarray(gxbias),
        "h0T": h0T,
        "lo": lo,
        "hi": hi,
    }
    bhhn = bhh[2 * H :]
    if np.any(bhhn != 0):
        m["bhhn"] = np.ascontiguousarray(
            np.broadcast_to(
                bhhn.reshape(KT, 128).T[:, :, None], (128, KT, BL)
            ).reshape(128, GC)
        ).astype(np.float32)
    return m


def make_in_maps(inputs, t_steps=T):
    x = np.asarray(inputs["x"], np.float32)
    h0 = np.asarray(inputs["h0"], np.float32)
    lengths = np.asarray(inputs["lengths"])
    maps = []
    any_bhhn = False
    for core in range(8):
        direction = core // 4
        sl = core % 4
        wih = inputs["wih_f"] if direction == 0 else inputs["wih_b"]
        whh = inputs["whh_f"] if direction == 0 else inputs["whh_b"]
        bih = inputs["bih_f"] if direction == 0 else inputs["bih_b"]
        bhh = inputs["bhh_f"] if direction == 0 else inputs["bhh_b"]
        m = _prep_core_inputs(
            x, h0,
            np.asarray(wih, np.float32), np.asarray(whh, np.float32),
            np.asarray(bih, np.float32), np.asarray(bhh, np.float32),
            lengths, direction, sl, t_steps,
        )
        any_bhhn = any_bhhn or ("bhhn" in m)
        maps.append(m)
    if any_bhhn:
        for m in maps:
            if "bhhn" not in m:
                m["bhhn"] = np.zeros((128, GC), np.float32)
    return maps, any_bhhn


def assemble_outputs(results, lengths, t_steps=T):
    """t_steps may be < T (time loop truncated to max(lengths) rounded up);
    rows beyond t_steps are all masked-out and stay zero."""
    out = np.zeros((T, B, 2 * H), np.float32)
    h_out = np.zeros((2, B, H), np.float32)
    for core in range(8):
        direction = core // 4
        sl = core % 4
        bsl = slice(sl * BL, (sl + 1) * BL)
        stage = results[core]["ostage"].reshape(128, t_steps, KT, BL)
        if direction == 1:
            stage = stage[:, ::-1]
        # out[t, b, k*128+p] = stage[p, t, k, b]
        blockv = stage.transpose(1, 3, 2, 0).reshape(t_steps, BL, H)
        out[:t_steps, bsl, direction * H : (direction + 1) * H] = blockv
        hl = results[core]["hlast"].reshape(128, KT, BL)
        h_out[direction, bsl, :] = hl.transpose(2, 1, 0).reshape(BL, H)
    tmask = np.arange(T)[:, None] >= np.asarray(lengths)[None, :]
    out[tmask] = 0.0
    return out, h_out


_prog_cache = {}


def kernel(**inputs):
    from concourse.bass_utils import run_bass_kernel_spmd

    # Steps beyond max(lengths) are frozen for every sequence in both
    # directions — truncate the compiled time loop (the program is built
    # after lengths are known) and zero-pad on the host.
    lmax = int(np.max(np.asarray(inputs["lengths"])))
    t_steps = min(T, max(16, ((lmax + 15) // 16) * 16))
    maps, any_bhhn = make_in_maps(inputs, t_steps=t_steps)
    key = (t_steps, any_bhhn)
    if key not in _prog_cache:
        _prog_cache[key] = build_program(t_steps, any_bhhn)
    nc = _prog_cache[key]
    res = run_bass_kernel_spmd(nc, maps, list(range(8)))
    out, h_out = assemble_outputs(res.results, inputs["lengths"], t_steps=t_steps)
    return out, h_out
